# revision 19
# baseline (speedup 1.0000x reference)
"""Trainium2 Bass kernel for nn_AttentionModel (sparse banded attention).

Math (per batch element, data-parallel over 8 cores):
  qs    = q @ W_score.T
  score = qs @ k.T                      # only the 129-wide causal band matters
  w     = banded_softmax(score)         # full-row max cancels mathematically
  c     = w @ k
  enh   = tanh(concat([c, q]) @ W_enh.T + b_enh)
  out   = sigmoid(enh @ W_mask.T + b_mask)

Implementation notes (v3):
  - T=2000 padded: keys get 128 zero rows in front + 48 tail -> 2176 = 17*128;
    queries get 48 tail pad -> 2048 = 16*128.  Query tile j attends key blocks
    j (prev) and j+1 (diag) of the padded key array.
  - Score path (P0 + scores) stays fp32r: exact, and at >=256 moving columns
    fp32r streams 1 cycle/row -- same PE speed as bf16.
  - The band mask is applied by the DVE (PSUM scores + mask -> SBUF), off
    the PE.
  - Softmax skips max subtraction (cancels exactly; in-band |score|<~60).
    exp on ACT gives row sums via accum_out; normalize on the DVE.
  - w transposes run on the PE in bf16 (1 cycle/row vs fp32r's 1.5).
    PV runs in bf16 with the pair scheme: per query-tile pair the
    shared middle key block does one 256-wide matmul, the two edge blocks do
    128-wide matmuls (bf16 has no sub-256 penalty).
  - P2 splits: the q-half is exact fp32r; the c-half runs as one fp8e4m3
    DoubleRow matmul (both 128-row contraction tiles in one instruction).
    Only c and W_enh[:, :256] are ever quantized to fp8; max rel err ~1.3e-2
    (sim-verified) vs the 2e-2 gate.
  - P3 is bf16 with b_mask preloaded into PSUM (DVE), so ACT applies
    tanh(0.5*z) straight from PSUM; sigmoid(x) = 0.5*tanh(0.5x)+0.5 keeps ACT
    on one table set (exp+tanh).
  - The attention loop is software-pipelined with a lag of 3 tiles; input DMA
    is spread over the sync/scalar/gpsimd queues.
"""

import sys
import types

import numpy as np
import ml_dtypes
from contextlib import ExitStack

import concourse.bass as bass
import concourse.bacc as bacc
import concourse.tile as tile
from concourse import mybir
from concourse.bass_utils import run_bass_kernel_spmd


def _ensure_axon_hooks():
    # bass_utils imports antenv.axon_hooks when tracing is requested; some
    # images lack that module.  Register a shim built from the boot helper
    # so a BASS_TRACE=1 environment doesn't crash the kernel.
    try:
        from antenv import axon_hooks  # noqa: F401
        return
    except ImportError:
        pass
    try:
        from trn_agent_boot.trn_boot import _ntff_profile_via_ctypes
        hook = _ntff_profile_via_ctypes("/opt/axon/libaxon_pjrt.so")
    except Exception:
        hook = None
    m = types.ModuleType("antenv.axon_hooks")
    m.get_axon_ntff_profile_hook = lambda: hook
    m.set_axon_ntff_profile_hook = lambda h: None
    sys.modules["antenv.axon_hooks"] = m


_ensure_axon_hooks()

F32 = mybir.dt.float32
F32R = mybir.dt.float32r
BF16 = mybir.dt.bfloat16
FP8 = mybir.dt.float8e4
AF = mybir.ActivationFunctionType
ALU = mybir.AluOpType
DRM = mybir.MatmulPerfMode.DoubleRow

NP_BF16 = ml_dtypes.bfloat16
NP_FP8 = ml_dtypes.float8_e4m3

B, T, H, F_OUT = 8, 2000, 256, 257
TPK = 2176   # padded key length   (128 front + 2000 + 48 tail)
TPQ = 2048   # padded query length (2000 + 48 tail)
NT = 16      # query tiles of 128
NKB = 17     # key blocks of 128
NEG = -32768.0
OPAD = 258   # F_OUT padded even
N_CORES = 8

_CACHE = {}


def _consts():
    t_i = np.arange(128, dtype=np.int32)[:, None]
    s_i = np.arange(128, dtype=np.int32)[None, :]
    mask_prev = np.where(s_i >= t_i, 0.0, NEG).astype(np.float32)
    mask_diag = np.where(s_i <= t_i, 0.0, NEG).astype(np.float32)
    mask_std = np.ascontiguousarray(np.concatenate([mask_prev, mask_diag], 1))
    mask_t0 = np.ascontiguousarray(
        np.concatenate([np.full((128, 128), NEG, np.float32), mask_diag], 1)
    )
    return mask_std, mask_t0


def build_nc():
    nc = bacc.Bacc("TRN2", target_bir_lowering=False, debug=False,
                   num_devices=N_CORES)

    kT = nc.declare_dram_parameter("kT", [H, TPK], F32R, isOutput=False)
    qT = nc.declare_dram_parameter("qT", [H, TPQ], F32R, isOutput=False)
    kN16 = nc.declare_dram_parameter("kN16", [128, NKB * 256], BF16,
                                     isOutput=False)
    WsT = nc.declare_dram_parameter("WsT", [H, H], F32R, isOutput=False)
    WeqT = nc.declare_dram_parameter("WeqT", [H, H], F32R, isOutput=False)
    Wec8 = nc.declare_dram_parameter("Wec8", [128, 2 * H], FP8, isOutput=False)
    WmT16 = nc.declare_dram_parameter("WmT16", [H, OPAD], BF16, isOutput=False)
    be = nc.declare_dram_parameter("be", [H, 1], F32, isOutput=False)
    bm = nc.declare_dram_parameter("bm", [128, OPAD], F32, isOutput=False)
    out = nc.declare_dram_parameter("out", [T, F_OUT], F32, isOutput=True)
    scr = nc.declare_dram_parameter("scr", [1, 4], F32, isOutput=True)

    mask_std_np, mask_t0_np = _consts()
    mask_std_d = nc.inline_tensor(mask_std_np, "mask_stdc")
    mask_t0_d = nc.inline_tensor(mask_t0_np, "mask_t0c")
    identu_np = (np.eye(128, dtype=np.uint16) * 0x3F80).astype(np.uint16)
    identu_d = nc.inline_tensor(identu_np, "identc")

    with tile.TileContext(nc) as tc, ExitStack() as ctx:
        const = ctx.enter_context(tc.tile_pool(name="const", bufs=1))
        io = ctx.enter_context(tc.tile_pool(name="io", bufs=1))
        wk = ctx.enter_context(tc.tile_pool(name="wk", bufs=4))
        stat = ctx.enter_context(tc.tile_pool(name="stat", bufs=8))
        pmm = ctx.enter_context(tc.tile_pool(name="pmm", bufs=2, space="PSUM"))
        psc = ctx.enter_context(tc.tile_pool(name="psc", bufs=3, space="PSUM"))
        pct = ctx.enter_context(tc.tile_pool(name="pct", bufs=2, space="PSUM"))
        pwt = ctx.enter_context(tc.tile_pool(name="pwt", bufs=1, space="PSUM"))

        def cload(tag, shape, src, dt, q=nc.gpsimd):
            t = const.tile(shape, dt, tag=tag, name=tag)
            q.dma_start(t[:], src)
            return t

        # DMA is ordered by first need, spread over 3 rings sharing HBM:
        #   scalar: wst -> qT(nb0) -> masks/ident -> qT rest -> P2 consts
        #   sync:   kT column chunks (scores need window j*128 early)
        #   gpsimd: kN in pair-order chunks -> P3 consts
        wst = [cload(f"wst{c}", [128, H], WsT[c * 128:(c + 1) * 128, :], F32R,
                     q=nc.scalar)
               for c in range(2)]
        qT_t = [io.tile([128, TPQ], F32R, tag=f"qT{c}", name=f"qT{c}")
                for c in range(2)]

        def load_qt(nb):
            for c in range(2):
                nc.scalar.dma_start(
                    qT_t[c][:, nb * 512:(nb + 1) * 512],
                    qT[c * 128:(c + 1) * 128, nb * 512:(nb + 1) * 512])

        load_qt(0)
        mask_std = cload("mask_std", [128, 256], mask_std_d[:], F32,
                         q=nc.scalar)
        mask_t0 = cload("mask_t0", [128, 256], mask_t0_d[:], F32, q=nc.scalar)
        identu_t = cload("ident", [128, 128], identu_d[:], mybir.dt.uint16,
                         q=nc.scalar)
        ident = identu_t[:].bitcast(BF16)
        for nb in range(1, 4):
            load_qt(nb)
        weq = [cload(f"weq{d}", [128, H], WeqT[d * 128:(d + 1) * 128, :], F32R,
                     q=nc.scalar)
               for d in range(2)]
        wec8 = cload("wec8", [128, 2 * H], Wec8[:], FP8, q=nc.scalar)
        bet = [cload(f"bet{f}", [128, 1], be[f * 128:(f + 1) * 128, :], F32,
                     q=nc.scalar)
               for f in range(2)]

        kT_t = [io.tile([128, TPK], F32R, tag=f"kT{c}", name=f"kT{c}")
                for c in range(2)]
        kN_t = io.tile([128, NKB * 256], BF16, tag="kN", name="kN_t")
        dummy = stat.tile([1, 1], F32, tag="dummy", name="dummy")

        def load_kt(i):
            for c in range(2):
                nc.sync.dma_start(
                    kT_t[c][:, i * 544:(i + 1) * 544],
                    kT[c * 128:(c + 1) * 128, i * 544:(i + 1) * 544])

        def load_kn(b0, b1):
            nc.gpsimd.dma_start(kN_t[:, b0 * 256: b1 * 256],
                                kN16[:, b0 * 256: b1 * 256])

        for i in range(4):
            load_kt(i)
        for b0, b1 in ((0, 5), (5, 9), (9, 13), (13, 17)):
            load_kn(b0, b1)
        wmt = [cload(f"wmt{f}", [128, OPAD],
                     WmT16[f * 128:(f + 1) * 128, :], BF16)
               for f in range(2)]
        bm_t = cload("bm", [128, OPAD], bm[:], F32)

        qsT_t = [io.tile([128, TPQ], F32R, tag=f"qsT{c}", name=f"qsT{c}")
                 for c in range(2)]
        c8_t = io.tile([128, 2 * TPQ], FP8, tag="c8", name="c8_t")
        enh_t = io.tile([128, 2 * TPQ], BF16, tag="enh", name="enh_t")
        # transposed softmax weights: per tile j cols [j*256, j*256+256) =
        # [prev-block | diag-block], each [s' 128, t' 128]
        wTall = io.tile([128, NT * 256], BF16, tag="wTall", name="wTall")

        kNv = kN_t[:].rearrange("p (b x) -> p b x", x=256)       # [128,17,256]
        c8v = c8_t[:].rearrange("p (i x) -> p i x", x=TPQ)       # [128,2,2048]
        wecv = wec8[:].rearrange("p (i f) -> p i f", f=H)        # [128,2,256]
        env = enh_t[:].rearrange("p (i x) -> p i x", x=TPQ)      # [128,2,2048]

        # ---- P0: qsT[g, t'] = (q @ W_score.T).T  (fp32r) ----
        # nb-major so tile-0 columns land first; copies alternate ACT/DVE
        def p0(nb):
            for c in range(2):      # g chunk (psum partition dim)
                ps = pmm.tile([128, 512], F32, tag="mm", name="ps")
                for h in range(2):  # contraction chunk
                    nc.tensor.matmul(
                        ps[:],
                        wst[h][:, c * 128:(c + 1) * 128],
                        qT_t[h][:, nb * 512:(nb + 1) * 512],
                        start=(h == 0), stop=(h == 1))
                dst = qsT_t[c][:, nb * 512:(nb + 1) * 512]
                if c == 0:
                    nc.scalar.copy(dst, ps[:])
                else:
                    nc.vector.tensor_copy(dst, ps[:])

        # ---- per-tile attention stages ----
        def scores(j):
            ps = psc.tile([128, 256], F32, tag="sc", name="ps")
            for c in range(2):
                nc.tensor.matmul(
                    ps[:],
                    qsT_t[c][:, j * 128:(j + 1) * 128],
                    kT_t[c][:, j * 128: j * 128 + 256],
                    start=(c == 0), stop=(c == 1))
            return ps

        def softmax(j, ps):
            # band mask on DVE, then exp (no max subtraction needed) with
            # free row sums; normalize on gpsimd; bf16 weights
            scm = wk.tile([128, 256], F32, tag="scm", name="scm")
            nc.vector.tensor_add(scm[:], ps[:],
                                 (mask_t0 if j == 0 else mask_std)[:])
            e_t = wk.tile([128, 256], BF16, tag="e", name="e_t")
            den = stat.tile([128, 1], F32, tag="den", name="den")
            nc.scalar.activation(e_t[:], scm[:], AF.Exp, accum_out=den[:])
            rec = stat.tile([128, 1], F32, tag="rec", name="rec")
            nc.vector.reciprocal(rec[:], den[:])
            w_t = wk.tile([128, 256], BF16, tag="w", name="w_t")
            nc.vector.tensor_scalar_mul(w_t[:], e_t[:], rec[:])
            # transpose both 128-wide halves on the PE (bf16, 1 cyc/row)
            pw = pwt.tile([128, 256], BF16, tag="pw", name="pw")
            nc.tensor.transpose(pw[:, 0:128], w_t[:, 0:128], ident)
            nc.tensor.transpose(pw[:, 128:256], w_t[:, 128:256], ident)
            nc.vector.tensor_copy(wTall[:, j * 256:(j + 1) * 256], pw[:])

        def pv(p):
            # pair PV in bf16: middle key block shared by both tiles
            # (256-wide moving), edge blocks 128-wide
            pc = pct.tile([128, 512], F32, tag="pc", name="pc")
            base = 2 * p * 256
            for h in range(2):
                hs = slice(h * 128, (h + 1) * 128)
                o = h * 256
                nc.tensor.matmul(          # m=1: key block 2p+1, both tiles
                    pc[:, o: o + 256],
                    kNv[:, 2 * p + 1, hs],
                    wTall[:, base + 128: base + 384],
                    start=True, stop=False)
                nc.tensor.matmul(          # m=0: key block 2p, tile 2p only
                    pc[:, o: o + 128],
                    kNv[:, 2 * p, hs],
                    wTall[:, base: base + 128],
                    start=False, stop=False, skip_group_check=True)
                nc.tensor.matmul(          # m=2: key block 2p+2, tile 2p+1
                    pc[:, o + 128: o + 256],
                    kNv[:, 2 * p + 2, hs],
                    wTall[:, base + 384: base + 512],
                    start=False, stop=True, skip_group_check=True)
            # scatter h-chunks into c8 slots (stride TPQ), cast to fp8
            nc.vector.tensor_copy(
                c8v[:, 0:2, 2 * p * 128: 2 * p * 128 + 256],
                pc[:].rearrange("p (b x) -> p b x", x=256))

        def p2(nb, half=None):
            # enhT[f, t'] = tanh(W_enh.T @ [cT; qT] + b_enh)
            # q-half exact fp32r; c-half one fp8 DoubleRow matmul
            if half is None:
                t0, tw = nb * 512, 512
            else:
                t0, tw = nb * 512 + half * 256, 256
            for f in range(2):
                pe_ = pmm.tile([128, tw], F32, tag="mm", name="pe_")
                for d in range(2):
                    nc.tensor.matmul(
                        pe_[:],
                        weq[d][:, f * 128:(f + 1) * 128],
                        qT_t[d][:, t0:t0 + tw],
                        start=(d == 0), stop=False)
                nc.tensor.matmul(
                    pe_[:],
                    wecv[:, 0:2, f * 128:(f + 1) * 128],
                    c8v[:, 0:2, t0:t0 + tw],
                    start=False, stop=True, perf_mode=DRM)
                nc.scalar.activation(
                    env[:, f:f + 1, t0:t0 + tw],
                    pe_[:].rearrange("p (b x) -> p b x", x=tw),
                    AF.Tanh, bias=bet[f][:, 0:1])

        def p3(j):
            # z = enh @ W_mask.T + b_mask ; out = 0.5*tanh(z/2)+0.5
            pm = pmm.tile([128, OPAD], F32, tag="mm", name="pm")
            for f in range(2):
                nc.tensor.matmul(
                    pm[:],
                    env[:, f:f + 1, j * 128:(j + 1) * 128],
                    wmt[f][:],
                    start=(f == 0), stop=(f == 1))
            z_t = wk.tile([128, OPAD], F32, tag="z", name="z_t")
            nc.vector.tensor_add(z_t[:], pm[:], bm_t[:])
            o_t = wk.tile([128, OPAD], F32, tag="o", name="o_t")
            nc.scalar.activation(o_t[:], z_t[:], AF.Tanh, scale=0.5)
            o2_t = wk.tile([128, OPAD], F32, tag="o2", name="o2_t")
            nc.gpsimd.tensor_scalar(o2_t[:], o_t[:], 0.5, 0.5,
                                    op0=ALU.mult, op1=ALU.add)
            rows = min(128, T - j * 128)
            nc.sync.dma_start(out[j * 128: j * 128 + rows, :],
                              o2_t[0:rows, 0:F_OUT])

        # ---- attention loop, software-pipelined with lag 2 ----
        # P3 of group g runs spread over group g+1's iterations
        LAG = 2
        p0(0)
        ps_q = {jj: scores(jj) for jj in range(LAG)}
        for nb in range(1, 4):
            p0(nb)
        pending_p3 = []
        for j in range(NT):
            if j % 4 == 0 and j > 0:
                nb = j // 4 - 1
                p2(nb)
                pending_p3.extend(range(nb * 4, nb * 4 + 4))
            if j + LAG < NT:
                ps_q[j + LAG] = scores(j + LAG)
            softmax(j, ps_q.pop(j))
            if j == 14:
                p2(3, half=0)      # tiles 12,13 (pair 6 stored at j=13)
                p3(12)
                p3(13)
            if pending_p3:
                p3(pending_p3.pop(0))
            if j % 2 == 1:
                pv(j // 2)
        p2(3, half=1)              # tiles 14,15
        for jj in [14, 15]:
            p3(jj)

    return nc


def _prep_shared(W_score, W_enh, b_enh, W_mask, b_mask):
    WsT = np.ascontiguousarray(W_score.T.astype(np.float32))        # [h, g]
    We = np.ascontiguousarray(W_enh.T.astype(np.float32))           # [d, f]
    WeqT = np.ascontiguousarray(We[H:])                             # [d', f]
    Wec8 = np.ascontiguousarray(
        We[:H].reshape(2, 128, H).transpose(1, 0, 2).reshape(128, 2 * H)
    ).astype(NP_FP8)
    Wm = np.zeros((H, OPAD), np.float32)                            # [f, o]
    Wm[:, :F_OUT] = W_mask.T.astype(np.float32)
    WmT16 = Wm.astype(NP_BF16)
    be = np.ascontiguousarray(b_enh.astype(np.float32).reshape(H, 1))
    bmv = np.zeros((128, OPAD), np.float32)
    bmv[:, :F_OUT] = b_mask.astype(np.float32)[None, :]
    return WsT, WeqT, Wec8, WmT16, be, bmv


def make_in_maps(k, q, W_score, W_enh, b_enh, W_mask, b_mask):
    k = np.asarray(k, np.float32)
    q = np.asarray(q, np.float32)
    WsT, WeqT, Wec8, WmT16, be, bmv = _prep_shared(
        np.asarray(W_score, np.float32), np.asarray(W_enh, np.float32),
        np.asarray(b_enh, np.float32), np.asarray(W_mask, np.float32),
        np.asarray(b_mask, np.float32))
    in_maps = []
    for b in range(N_CORES):
        kb = np.zeros((TPK, H), np.float32)
        kb[128:128 + T] = k[b]
        qb = np.zeros((TPQ, H), np.float32)
        qb[:T] = q[b]
        kN16 = np.ascontiguousarray(
            kb.reshape(NKB, 128, H).transpose(1, 0, 2).reshape(128, NKB * 256)
        ).astype(NP_BF16)
        in_maps.append({
            "kT": np.ascontiguousarray(kb.T),
            "qT": np.ascontiguousarray(qb.T),
            "kN16": kN16,
            "WsT": WsT, "WeqT": WeqT, "Wec8": Wec8, "WmT16": WmT16,
            "be": be, "bm": bmv,
        })
    return in_maps


def get_nc():
    if "nc" not in _CACHE:
        nc = build_nc()
        nc.finalize()
        _CACHE["nc"] = nc
    return _CACHE["nc"]


def kernel(k, q, W_score, W_enh, b_enh, W_mask, b_mask):
    in_maps = make_in_maps(k, q, W_score, W_enh, b_enh, W_mask, b_mask)
    res = run_bass_kernel_spmd(get_nc(), in_maps, list(range(N_CORES)))
    return np.stack([r["out"] for r in res.results], 0)


# revision 20
# speedup vs baseline: 1.1348x; 1.1348x over previous
"""Trainium2 Bass kernel for nn_AttentionModel (sparse banded attention).

Math (per batch element, data-parallel over 8 cores):
  qs    = q @ W_score.T
  score = qs @ k.T                      # only the 129-wide causal band matters
  w     = banded_softmax(score)         # full-row max cancels mathematically
  c     = w @ k
  enh   = tanh(concat([c, q]) @ W_enh.T + b_enh)
  out   = sigmoid(enh @ W_mask.T + b_mask)

Implementation notes (v3):
  - T=2000 padded: keys get 128 zero rows in front + 48 tail -> 2176 = 17*128;
    queries get 48 tail pad -> 2048 = 16*128.  Query tile j attends key blocks
    j (prev) and j+1 (diag) of the padded key array.
  - Score path (P0 + scores) stays fp32r: exact, and at >=256 moving columns
    fp32r streams 1 cycle/row -- same PE speed as bf16.
  - The band mask is applied by the DVE (PSUM scores + mask -> SBUF), off
    the PE.
  - Softmax skips max subtraction (cancels exactly; in-band |score|<~60).
    exp on ACT gives row sums via accum_out; normalize on the DVE.
  - w transposes run on the PE in bf16 (1 cycle/row vs fp32r's 1.5).
    PV runs in bf16 with the pair scheme: per query-tile pair the
    shared middle key block does one 256-wide matmul, the two edge blocks do
    128-wide matmuls (bf16 has no sub-256 penalty).
  - P2 splits: the q-half is exact fp32r; the c-half runs as one fp8e4m3
    DoubleRow matmul (both 128-row contraction tiles in one instruction).
    Only c and W_enh[:, :256] are ever quantized to fp8; max rel err ~1.3e-2
    (sim-verified) vs the 2e-2 gate.
  - P3 is bf16 with b_mask preloaded into PSUM (DVE), so ACT applies
    tanh(0.5*z) straight from PSUM; sigmoid(x) = 0.5*tanh(0.5x)+0.5 keeps ACT
    on one table set (exp+tanh).
  - The attention loop is software-pipelined with a lag of 3 tiles; input DMA
    is spread over the sync/scalar/gpsimd queues.
"""

import sys
import types

import numpy as np
import ml_dtypes
from contextlib import ExitStack

import concourse.bass as bass
import concourse.bacc as bacc
import concourse.tile as tile
from concourse import mybir
from concourse.bass_utils import run_bass_kernel_spmd


def _ensure_axon_hooks():
    # bass_utils imports antenv.axon_hooks when tracing is requested; some
    # images lack that module.  Register a shim built from the boot helper
    # so a BASS_TRACE=1 environment doesn't crash the kernel.
    try:
        from antenv import axon_hooks  # noqa: F401
        return
    except ImportError:
        pass
    try:
        from trn_agent_boot.trn_boot import _ntff_profile_via_ctypes
        hook = _ntff_profile_via_ctypes("/opt/axon/libaxon_pjrt.so")
    except Exception:
        hook = None
    m = types.ModuleType("antenv.axon_hooks")
    m.get_axon_ntff_profile_hook = lambda: hook
    m.set_axon_ntff_profile_hook = lambda h: None
    sys.modules["antenv.axon_hooks"] = m


_ensure_axon_hooks()

F32 = mybir.dt.float32
F32R = mybir.dt.float32r
BF16 = mybir.dt.bfloat16
FP8 = mybir.dt.float8e4
AF = mybir.ActivationFunctionType
ALU = mybir.AluOpType
DRM = mybir.MatmulPerfMode.DoubleRow

NP_BF16 = ml_dtypes.bfloat16
NP_FP8 = ml_dtypes.float8_e4m3

B, T, H, F_OUT = 8, 2000, 256, 257
TPK = 2176   # padded key length   (128 front + 2000 + 48 tail)
TPQ = 2048   # padded query length (2000 + 48 tail)
NT = 16      # query tiles of 128
NKB = 17     # key blocks of 128
NEG = -32768.0
OPAD = 258   # F_OUT padded even
N_CORES = 8

_CACHE = {}


def _consts():
    t_i = np.arange(128, dtype=np.int32)[:, None]
    s_i = np.arange(128, dtype=np.int32)[None, :]
    mask_prev = np.where(s_i >= t_i, 0.0, NEG).astype(np.float32)
    mask_diag = np.where(s_i <= t_i, 0.0, NEG).astype(np.float32)
    mask_std = np.ascontiguousarray(np.concatenate([mask_prev, mask_diag], 1))
    mask_t0 = np.ascontiguousarray(
        np.concatenate([np.full((128, 128), NEG, np.float32), mask_diag], 1)
    )
    return mask_std, mask_t0


def build_nc():
    nc = bacc.Bacc("TRN2", target_bir_lowering=False, debug=False,
                   num_devices=N_CORES)

    kT = nc.declare_dram_parameter("kT", [H, TPK], F32R, isOutput=False)
    qT = nc.declare_dram_parameter("qT", [H, TPQ], F32R, isOutput=False)
    kN16 = nc.declare_dram_parameter("kN16", [128, NKB * 256], BF16,
                                     isOutput=False)
    WsT = nc.declare_dram_parameter("WsT", [H, H], F32R, isOutput=False)
    WeqT = nc.declare_dram_parameter("WeqT", [H, H], F32R, isOutput=False)
    Wec8 = nc.declare_dram_parameter("Wec8", [128, 2 * H], FP8, isOutput=False)
    WmT16 = nc.declare_dram_parameter("WmT16", [H, OPAD], BF16, isOutput=False)
    be = nc.declare_dram_parameter("be", [H, 1], F32, isOutput=False)
    bm = nc.declare_dram_parameter("bm", [128, OPAD], F32, isOutput=False)
    out = nc.declare_dram_parameter("out", [T, F_OUT], F32, isOutput=True)
    scr = nc.declare_dram_parameter("scr", [1, 4], F32, isOutput=True)

    mask_std_np, mask_t0_np = _consts()
    mask_std_d = nc.inline_tensor(mask_std_np, "mask_stdc")
    mask_t0_d = nc.inline_tensor(mask_t0_np, "mask_t0c")
    identu_np = (np.eye(128, dtype=np.uint16) * 0x3F80).astype(np.uint16)
    identu_d = nc.inline_tensor(identu_np, "identc")

    with tile.TileContext(nc) as tc, ExitStack() as ctx:
        const = ctx.enter_context(tc.tile_pool(name="const", bufs=1))
        io = ctx.enter_context(tc.tile_pool(name="io", bufs=1))
        wk = ctx.enter_context(tc.tile_pool(name="wk", bufs=4))
        stat = ctx.enter_context(tc.tile_pool(name="stat", bufs=8))
        pmm = ctx.enter_context(tc.tile_pool(name="pmm", bufs=2, space="PSUM"))
        psc = ctx.enter_context(tc.tile_pool(name="psc", bufs=3, space="PSUM"))
        pct = ctx.enter_context(tc.tile_pool(name="pct", bufs=2, space="PSUM"))
        pwt = ctx.enter_context(tc.tile_pool(name="pwt", bufs=1, space="PSUM"))

        def cload(tag, shape, src, dt, q=nc.gpsimd):
            t = const.tile(shape, dt, tag=tag, name=tag)
            q.dma_start(t[:], src)
            return t

        # DMA is ordered by first need, spread over 3 rings sharing HBM:
        #   scalar: wst -> qT(nb0) -> masks/ident -> qT rest -> P2 consts
        #   sync:   kT column chunks (scores need window j*128 early)
        #   gpsimd: kN in pair-order chunks -> P3 consts
        wst = [cload(f"wst{c}", [128, H], WsT[c * 128:(c + 1) * 128, :], F32R,
                     q=nc.scalar)
               for c in range(2)]
        qT_t = [io.tile([128, TPQ], F32R, tag=f"qT{c}", name=f"qT{c}")
                for c in range(2)]

        def load_qt(nb):
            for c in range(2):
                nc.scalar.dma_start(
                    qT_t[c][:, nb * 512:(nb + 1) * 512],
                    qT[c * 128:(c + 1) * 128, nb * 512:(nb + 1) * 512])

        load_qt(0)
        mask_std = cload("mask_std", [128, 256], mask_std_d[:], F32,
                         q=nc.scalar)
        mask_t0 = cload("mask_t0", [128, 256], mask_t0_d[:], F32, q=nc.scalar)
        identu_t = cload("ident", [128, 128], identu_d[:], mybir.dt.uint16,
                         q=nc.scalar)
        ident = identu_t[:].bitcast(BF16)
        for nb in range(1, 4):
            load_qt(nb)
        weq = [cload(f"weq{d}", [128, H], WeqT[d * 128:(d + 1) * 128, :], F32R,
                     q=nc.scalar)
               for d in range(2)]
        wec8 = cload("wec8", [128, 2 * H], Wec8[:], FP8, q=nc.scalar)
        bet = [cload(f"bet{f}", [128, 1], be[f * 128:(f + 1) * 128, :], F32,
                     q=nc.scalar)
               for f in range(2)]

        kT_t = [io.tile([128, TPK], F32R, tag=f"kT{c}", name=f"kT{c}")
                for c in range(2)]
        kN_t = io.tile([128, NKB * 256], BF16, tag="kN", name="kN_t")
        dummy = stat.tile([1, 1], F32, tag="dummy", name="dummy")

        def load_kt(i):
            for c in range(2):
                nc.sync.dma_start(
                    kT_t[c][:, i * 544:(i + 1) * 544],
                    kT[c * 128:(c + 1) * 128, i * 544:(i + 1) * 544])

        def load_kn(b0, b1):
            nc.gpsimd.dma_start(kN_t[:, b0 * 256: b1 * 256],
                                kN16[:, b0 * 256: b1 * 256])

        for i in range(4):
            load_kt(i)
        for b0, b1 in ((0, 5), (5, 9), (9, 13), (13, 17)):
            load_kn(b0, b1)
        wmt = [cload(f"wmt{f}", [128, OPAD],
                     WmT16[f * 128:(f + 1) * 128, :], BF16)
               for f in range(2)]
        bm_t = cload("bm", [128, OPAD], bm[:], F32)

        qsT_t = [io.tile([128, TPQ], F32R, tag=f"qsT{c}", name=f"qsT{c}")
                 for c in range(2)]
        c8_t = io.tile([128, 2 * TPQ], FP8, tag="c8", name="c8_t")
        enh_t = io.tile([128, 2 * TPQ], BF16, tag="enh", name="enh_t")
        # transposed softmax weights: per tile j cols [j*256, j*256+256) =
        # [prev-block | diag-block], each [s' 128, t' 128]
        wTall = io.tile([128, NT * 256], BF16, tag="wTall", name="wTall")

        kNv = kN_t[:].rearrange("p (b x) -> p b x", x=256)       # [128,17,256]
        c8v = c8_t[:].rearrange("p (i x) -> p i x", x=TPQ)       # [128,2,2048]
        wecv = wec8[:].rearrange("p (i f) -> p i f", f=H)        # [128,2,256]
        env = enh_t[:].rearrange("p (i x) -> p i x", x=TPQ)      # [128,2,2048]

        # ---- P0: qsT[g, t'] = (q @ W_score.T).T  (fp32r) ----
        # nb-major so tile-0 columns land first; copies alternate ACT/DVE
        def p0(nb):
            for c in range(2):      # g chunk (psum partition dim)
                ps = pmm.tile([128, 512], F32, tag="mm", name="ps")
                for h in range(2):  # contraction chunk
                    nc.tensor.matmul(
                        ps[:],
                        wst[h][:, c * 128:(c + 1) * 128],
                        qT_t[h][:, nb * 512:(nb + 1) * 512],
                        start=(h == 0), stop=(h == 1))
                dst = qsT_t[c][:, nb * 512:(nb + 1) * 512]
                if c == 0:
                    nc.scalar.copy(dst, ps[:])
                else:
                    nc.vector.tensor_copy(dst, ps[:])

        # ---- per-tile attention stages ----
        def scores(j):
            ps = psc.tile([128, 256], F32, tag="sc", name="ps")
            for c in range(2):
                nc.tensor.matmul(
                    ps[:],
                    qsT_t[c][:, j * 128:(j + 1) * 128],
                    kT_t[c][:, j * 128: j * 128 + 256],
                    start=(c == 0), stop=(c == 1))
            return ps

        def softmax(j, ps):
            # band mask on DVE, then exp (no max subtraction needed) with
            # free row sums; normalize on gpsimd; bf16 weights
            scm = wk.tile([128, 256], F32, tag="scm", name="scm")
            nc.vector.tensor_add(scm[:], ps[:],
                                 (mask_t0 if j == 0 else mask_std)[:])
            e_t = wk.tile([128, 256], BF16, tag="e", name="e_t")
            den = stat.tile([128, 1], F32, tag="den", name="den")
            nc.scalar.activation(e_t[:], scm[:], AF.Exp, accum_out=den[:])
            rec = stat.tile([128, 1], F32, tag="rec", name="rec")
            nc.vector.reciprocal(rec[:], den[:])
            w_t = wk.tile([128, 256], BF16, tag="w", name="w_t")
            nc.vector.tensor_scalar_mul(w_t[:], e_t[:], rec[:])
            # transpose both 128-wide halves on the PE (bf16, 1 cyc/row)
            pw = pwt.tile([128, 256], BF16, tag="pw", name="pw")
            nc.tensor.transpose(pw[:, 0:128], w_t[:, 0:128], ident)
            nc.tensor.transpose(pw[:, 128:256], w_t[:, 128:256], ident)
            nc.vector.tensor_copy(wTall[:, j * 256:(j + 1) * 256], pw[:])

        def pv(p):
            # pair PV in bf16: middle key block shared by both tiles
            # (256-wide moving), edge blocks 128-wide
            pc = pct.tile([128, 512], F32, tag="pc", name="pc")
            base = 2 * p * 256
            for h in range(2):
                hs = slice(h * 128, (h + 1) * 128)
                o = h * 256
                nc.tensor.matmul(          # m=1: key block 2p+1, both tiles
                    pc[:, o: o + 256],
                    kNv[:, 2 * p + 1, hs],
                    wTall[:, base + 128: base + 384],
                    start=True, stop=False)
                nc.tensor.matmul(          # m=0: key block 2p, tile 2p only
                    pc[:, o: o + 128],
                    kNv[:, 2 * p, hs],
                    wTall[:, base: base + 128],
                    start=False, stop=False, skip_group_check=True)
                nc.tensor.matmul(          # m=2: key block 2p+2, tile 2p+1
                    pc[:, o + 128: o + 256],
                    kNv[:, 2 * p + 2, hs],
                    wTall[:, base + 384: base + 512],
                    start=False, stop=True, skip_group_check=True)
            # scatter h-chunks into c8 slots (stride TPQ), cast to fp8
            nc.vector.tensor_copy(
                c8v[:, 0:2, 2 * p * 128: 2 * p * 128 + 256],
                pc[:].rearrange("p (b x) -> p b x", x=256))

        def p2(nb, half=None):
            # enhT[f, t'] = tanh(W_enh.T @ [cT; qT] + b_enh)
            # q-half exact fp32r; c-half one fp8 DoubleRow matmul
            if half is None:
                t0, tw = nb * 512, 512
            else:
                t0, tw = nb * 512 + half * 256, 256
            for f in range(2):
                pe_ = pmm.tile([128, tw], F32, tag="mm", name="pe_")
                for d in range(2):
                    nc.tensor.matmul(
                        pe_[:],
                        weq[d][:, f * 128:(f + 1) * 128],
                        qT_t[d][:, t0:t0 + tw],
                        start=(d == 0), stop=False)
                nc.tensor.matmul(
                    pe_[:],
                    wecv[:, 0:2, f * 128:(f + 1) * 128],
                    c8v[:, 0:2, t0:t0 + tw],
                    start=False, stop=True, perf_mode=DRM)
                nc.scalar.activation(
                    env[:, f:f + 1, t0:t0 + tw],
                    pe_[:].rearrange("p (b x) -> p b x", x=tw),
                    AF.Tanh, bias=bet[f][:, 0:1])

        def p3(j):
            # z = enh @ W_mask.T + b_mask ; out = 0.5*tanh(z/2)+0.5
            pm = pmm.tile([128, OPAD], F32, tag="mm", name="pm")
            for f in range(2):
                nc.tensor.matmul(
                    pm[:],
                    env[:, f:f + 1, j * 128:(j + 1) * 128],
                    wmt[f][:],
                    start=(f == 0), stop=(f == 1))
            z_t = wk.tile([128, OPAD], F32, tag="z", name="z_t")
            nc.vector.tensor_add(z_t[:], pm[:], bm_t[:])
            o_t = wk.tile([128, OPAD], F32, tag="o", name="o_t")
            nc.scalar.activation(o_t[:], z_t[:], AF.Tanh, scale=0.5)
            o2_t = wk.tile([128, OPAD], F32, tag="o2", name="o2_t")
            nc.gpsimd.tensor_scalar(o2_t[:], o_t[:], 0.5, 0.5,
                                    op0=ALU.mult, op1=ALU.add)
            rows = min(128, T - j * 128)
            nc.sync.dma_start(out[j * 128: j * 128 + rows, :],
                              o2_t[0:rows, 0:F_OUT])

        # ---- attention loop, software-pipelined with lag 2 ----
        # P3 of group g runs spread over group g+1's iterations
        LAG = 2
        p0(0)
        ps_q = {jj: scores(jj) for jj in range(LAG)}
        for nb in range(1, 4):
            p0(nb)
        pending_p3 = []
        for j in range(NT):
            if j % 4 == 0 and j > 0:
                nb = j // 4 - 1
                p2(nb)
                pending_p3.extend(range(nb * 4, nb * 4 + 4))
            if j + LAG < NT:
                ps_q[j + LAG] = scores(j + LAG)
            softmax(j, ps_q.pop(j))
            if pending_p3:
                p3(pending_p3.pop(0))
            if j % 2 == 1:
                pv(j // 2)
        p2(3)
        for jj in pending_p3 + list(range(12, 16)):
            p3(jj)

    return nc


def _prep_shared(W_score, W_enh, b_enh, W_mask, b_mask):
    WsT = np.ascontiguousarray(W_score.T.astype(np.float32))        # [h, g]
    We = np.ascontiguousarray(W_enh.T.astype(np.float32))           # [d, f]
    WeqT = np.ascontiguousarray(We[H:])                             # [d', f]
    Wec8 = np.ascontiguousarray(
        We[:H].reshape(2, 128, H).transpose(1, 0, 2).reshape(128, 2 * H)
    ).astype(NP_FP8)
    Wm = np.zeros((H, OPAD), np.float32)                            # [f, o]
    Wm[:, :F_OUT] = W_mask.T.astype(np.float32)
    WmT16 = Wm.astype(NP_BF16)
    be = np.ascontiguousarray(b_enh.astype(np.float32).reshape(H, 1))
    bmv = np.zeros((128, OPAD), np.float32)
    bmv[:, :F_OUT] = b_mask.astype(np.float32)[None, :]
    return WsT, WeqT, Wec8, WmT16, be, bmv


def make_in_maps(k, q, W_score, W_enh, b_enh, W_mask, b_mask):
    k = np.asarray(k, np.float32)
    q = np.asarray(q, np.float32)
    WsT, WeqT, Wec8, WmT16, be, bmv = _prep_shared(
        np.asarray(W_score, np.float32), np.asarray(W_enh, np.float32),
        np.asarray(b_enh, np.float32), np.asarray(W_mask, np.float32),
        np.asarray(b_mask, np.float32))
    in_maps = []
    for b in range(N_CORES):
        kb = np.zeros((TPK, H), np.float32)
        kb[128:128 + T] = k[b]
        qb = np.zeros((TPQ, H), np.float32)
        qb[:T] = q[b]
        kN16 = np.ascontiguousarray(
            kb.reshape(NKB, 128, H).transpose(1, 0, 2).reshape(128, NKB * 256)
        ).astype(NP_BF16)
        in_maps.append({
            "kT": np.ascontiguousarray(kb.T),
            "qT": np.ascontiguousarray(qb.T),
            "kN16": kN16,
            "WsT": WsT, "WeqT": WeqT, "Wec8": Wec8, "WmT16": WmT16,
            "be": be, "bm": bmv,
        })
    return in_maps


def get_nc():
    if "nc" not in _CACHE:
        nc = build_nc()
        nc.finalize()
        _CACHE["nc"] = nc
    return _CACHE["nc"]


def kernel(k, q, W_score, W_enh, b_enh, W_mask, b_mask):
    in_maps = make_in_maps(k, q, W_score, W_enh, b_enh, W_mask, b_mask)
    res = run_bass_kernel_spmd(get_nc(), in_maps, list(range(N_CORES)))
    return np.stack([r["out"] for r in res.results], 0)


# revision 21
# speedup vs baseline: 1.1460x; 1.0098x over previous
"""Trainium2 Bass kernel for nn_AttentionModel (sparse banded attention).

Math (per batch element, data-parallel over 8 cores):
  qs    = q @ W_score.T
  score = qs @ k.T                      # only the 129-wide causal band matters
  w     = banded_softmax(score)         # full-row max cancels mathematically
  c     = w @ k
  enh   = tanh(concat([c, q]) @ W_enh.T + b_enh)
  out   = sigmoid(enh @ W_mask.T + b_mask)

Implementation notes (v3):
  - T=2000 padded: keys get 128 zero rows in front + 48 tail -> 2176 = 17*128;
    queries get 48 tail pad -> 2048 = 16*128.  Query tile j attends key blocks
    j (prev) and j+1 (diag) of the padded key array.
  - Score path (P0 + scores) stays fp32r: exact, and at >=256 moving columns
    fp32r streams 1 cycle/row -- same PE speed as bf16.
  - The band mask is applied by the DVE (PSUM scores + mask -> SBUF), off
    the PE.
  - Softmax skips max subtraction (cancels exactly; in-band |score|<~60).
    exp on ACT gives row sums via accum_out; normalize on the DVE.
  - w transposes run on the PE in bf16 (1 cycle/row vs fp32r's 1.5).
    PV runs in bf16 with the pair scheme: per query-tile pair the
    shared middle key block does one 256-wide matmul, the two edge blocks do
    128-wide matmuls (bf16 has no sub-256 penalty).
  - P2 splits: the q-half is exact fp32r; the c-half runs as one fp8e4m3
    DoubleRow matmul (both 128-row contraction tiles in one instruction).
    Only c and W_enh[:, :256] are ever quantized to fp8; max rel err ~1.3e-2
    (sim-verified) vs the 2e-2 gate.
  - P3 is bf16 with b_mask preloaded into PSUM (DVE), so ACT applies
    tanh(0.5*z) straight from PSUM; sigmoid(x) = 0.5*tanh(0.5x)+0.5 keeps ACT
    on one table set (exp+tanh).
  - The attention loop is software-pipelined with a lag of 3 tiles; input DMA
    is spread over the sync/scalar/gpsimd queues.
"""

import sys
import types

import numpy as np
import ml_dtypes
from contextlib import ExitStack

import concourse.bass as bass
import concourse.bacc as bacc
import concourse.tile as tile
from concourse import mybir
from concourse.bass_utils import run_bass_kernel_spmd


def _ensure_axon_hooks():
    # bass_utils imports antenv.axon_hooks when tracing is requested; some
    # images lack that module.  Register a shim built from the boot helper
    # so a BASS_TRACE=1 environment doesn't crash the kernel.
    try:
        from antenv import axon_hooks  # noqa: F401
        return
    except ImportError:
        pass
    try:
        from trn_agent_boot.trn_boot import _ntff_profile_via_ctypes
        hook = _ntff_profile_via_ctypes("/opt/axon/libaxon_pjrt.so")
    except Exception:
        hook = None
    m = types.ModuleType("antenv.axon_hooks")
    m.get_axon_ntff_profile_hook = lambda: hook
    m.set_axon_ntff_profile_hook = lambda h: None
    sys.modules["antenv.axon_hooks"] = m


_ensure_axon_hooks()

F32 = mybir.dt.float32
F32R = mybir.dt.float32r
BF16 = mybir.dt.bfloat16
FP8 = mybir.dt.float8e4
AF = mybir.ActivationFunctionType
ALU = mybir.AluOpType
DRM = mybir.MatmulPerfMode.DoubleRow

NP_BF16 = ml_dtypes.bfloat16
NP_FP8 = ml_dtypes.float8_e4m3

B, T, H, F_OUT = 8, 2000, 256, 257
TPK = 2176   # padded key length   (128 front + 2000 + 48 tail)
TPQ = 2048   # padded query length (2000 + 48 tail)
NT = 16      # query tiles of 128
NKB = 17     # key blocks of 128
NEG = -32768.0
OPAD = 258   # F_OUT padded even
N_CORES = 8

_CACHE = {}


def _consts():
    t_i = np.arange(128, dtype=np.int32)[:, None]
    s_i = np.arange(128, dtype=np.int32)[None, :]
    mask_prev = np.where(s_i >= t_i, 0.0, NEG).astype(np.float32)
    mask_diag = np.where(s_i <= t_i, 0.0, NEG).astype(np.float32)
    mask_std = np.ascontiguousarray(np.concatenate([mask_prev, mask_diag], 1))
    mask_t0 = np.ascontiguousarray(
        np.concatenate([np.full((128, 128), NEG, np.float32), mask_diag], 1)
    )
    return mask_std, mask_t0


def build_nc():
    nc = bacc.Bacc("TRN2", target_bir_lowering=False, debug=False,
                   num_devices=N_CORES)

    kT = nc.declare_dram_parameter("kT", [H, TPK], F32R, isOutput=False)
    qT = nc.declare_dram_parameter("qT", [H, TPQ], F32R, isOutput=False)
    kN16 = nc.declare_dram_parameter("kN16", [128, NKB * 256], BF16,
                                     isOutput=False)
    WsT = nc.declare_dram_parameter("WsT", [H, H], F32R, isOutput=False)
    WeqT = nc.declare_dram_parameter("WeqT", [H, H], F32R, isOutput=False)
    Wec8 = nc.declare_dram_parameter("Wec8", [128, 2 * H], FP8, isOutput=False)
    WmT16 = nc.declare_dram_parameter("WmT16", [H, OPAD], BF16, isOutput=False)
    be = nc.declare_dram_parameter("be", [H, 1], F32, isOutput=False)
    bm = nc.declare_dram_parameter("bm", [128, OPAD], F32, isOutput=False)
    out = nc.declare_dram_parameter("out", [T, F_OUT], F32, isOutput=True)
    scr = nc.declare_dram_parameter("scr", [1, 4], F32, isOutput=True)

    mask_std_np, mask_t0_np = _consts()
    mask_std_d = nc.inline_tensor(mask_std_np, "mask_stdc")
    mask_t0_d = nc.inline_tensor(mask_t0_np, "mask_t0c")
    identu_np = (np.eye(128, dtype=np.uint16) * 0x3F80).astype(np.uint16)
    identu_d = nc.inline_tensor(identu_np, "identc")

    with tile.TileContext(nc) as tc, ExitStack() as ctx:
        const = ctx.enter_context(tc.tile_pool(name="const", bufs=1))
        io = ctx.enter_context(tc.tile_pool(name="io", bufs=1))
        wk = ctx.enter_context(tc.tile_pool(name="wk", bufs=4))
        stat = ctx.enter_context(tc.tile_pool(name="stat", bufs=8))
        pmm = ctx.enter_context(tc.tile_pool(name="pmm", bufs=2, space="PSUM"))
        psc = ctx.enter_context(tc.tile_pool(name="psc", bufs=3, space="PSUM"))
        pct = ctx.enter_context(tc.tile_pool(name="pct", bufs=2, space="PSUM"))
        pwt = ctx.enter_context(tc.tile_pool(name="pwt", bufs=1, space="PSUM"))

        def cload(tag, shape, src, dt, q=nc.gpsimd):
            t = const.tile(shape, dt, tag=tag, name=tag)
            q.dma_start(t[:], src)
            return t

        # DMA is ordered by first need, spread over 3 rings sharing HBM:
        #   scalar: wst -> qT(nb0) -> masks/ident -> qT rest -> P2 consts
        #   sync:   kT column chunks (scores need window j*128 early)
        #   gpsimd: kN in pair-order chunks -> P3 consts
        wst = [cload(f"wst{c}", [128, H], WsT[c * 128:(c + 1) * 128, :], F32R,
                     q=nc.scalar)
               for c in range(2)]
        qT_t = [io.tile([128, TPQ], F32R, tag=f"qT{c}", name=f"qT{c}")
                for c in range(2)]

        def load_qt(nb):
            for c in range(2):
                nc.scalar.dma_start(
                    qT_t[c][:, nb * 512:(nb + 1) * 512],
                    qT[c * 128:(c + 1) * 128, nb * 512:(nb + 1) * 512])

        load_qt(0)
        mask_std = cload("mask_std", [128, 256], mask_std_d[:], F32,
                         q=nc.scalar)
        mask_t0 = cload("mask_t0", [128, 256], mask_t0_d[:], F32, q=nc.scalar)
        identu_t = cload("ident", [128, 128], identu_d[:], mybir.dt.uint16,
                         q=nc.scalar)
        ident = identu_t[:].bitcast(BF16)
        for nb in range(1, 4):
            load_qt(nb)
        weq = [cload(f"weq{d}", [128, H], WeqT[d * 128:(d + 1) * 128, :], F32R,
                     q=nc.scalar)
               for d in range(2)]
        wec8 = cload("wec8", [128, 2 * H], Wec8[:], FP8, q=nc.scalar)
        bet = [cload(f"bet{f}", [128, 1], be[f * 128:(f + 1) * 128, :], F32,
                     q=nc.scalar)
               for f in range(2)]

        kT_t = [io.tile([128, TPK], F32R, tag=f"kT{c}", name=f"kT{c}")
                for c in range(2)]
        kN_t = io.tile([128, NKB * 256], BF16, tag="kN", name="kN_t")
        dummy = stat.tile([1, 1], F32, tag="dummy", name="dummy")

        def load_kt(i):
            for c in range(2):
                nc.sync.dma_start(
                    kT_t[c][:, i * 544:(i + 1) * 544],
                    kT[c * 128:(c + 1) * 128, i * 544:(i + 1) * 544])

        def load_kn(b0, b1):
            nc.gpsimd.dma_start(kN_t[:, b0 * 256: b1 * 256],
                                kN16[:, b0 * 256: b1 * 256])

        for i in range(4):
            load_kt(i)
        for b0, b1 in ((0, 5), (5, 9), (9, 13), (13, 17)):
            load_kn(b0, b1)
        wmt = [cload(f"wmt{f}", [128, OPAD],
                     WmT16[f * 128:(f + 1) * 128, :], BF16)
               for f in range(2)]
        bm_t = cload("bm", [128, OPAD], bm[:], F32)

        qsT_t = [io.tile([128, TPQ], F32R, tag=f"qsT{c}", name=f"qsT{c}")
                 for c in range(2)]
        c8_t = io.tile([128, 2 * TPQ], FP8, tag="c8", name="c8_t")
        enh_t = io.tile([128, 2 * TPQ], BF16, tag="enh", name="enh_t")
        # transposed softmax weights: per tile j cols [j*256, j*256+256) =
        # [prev-block | diag-block], each [s' 128, t' 128]
        wTall = io.tile([128, NT * 256], BF16, tag="wTall", name="wTall")

        kNv = kN_t[:].rearrange("p (b x) -> p b x", x=256)       # [128,17,256]
        c8v = c8_t[:].rearrange("p (i x) -> p i x", x=TPQ)       # [128,2,2048]
        wecv = wec8[:].rearrange("p (i f) -> p i f", f=H)        # [128,2,256]
        env = enh_t[:].rearrange("p (i x) -> p i x", x=TPQ)      # [128,2,2048]

        # ---- P0: qsT[g, t'] = (q @ W_score.T).T  (fp32r) ----
        # nb-major so tile-0 columns land first; copies alternate ACT/DVE
        def p0(nb):
            for c in range(2):      # g chunk (psum partition dim)
                ps = pmm.tile([128, 512], F32, tag="mm", name="ps")
                for h in range(2):  # contraction chunk
                    nc.tensor.matmul(
                        ps[:],
                        wst[h][:, c * 128:(c + 1) * 128],
                        qT_t[h][:, nb * 512:(nb + 1) * 512],
                        start=(h == 0), stop=(h == 1))
                dst = qsT_t[c][:, nb * 512:(nb + 1) * 512]
                if c == 0:
                    nc.scalar.copy(dst, ps[:])
                else:
                    nc.vector.tensor_copy(dst, ps[:])

        # ---- per-tile attention stages ----
        def scores(j):
            ps = psc.tile([128, 256], F32, tag="sc", name="ps")
            for c in range(2):
                nc.tensor.matmul(
                    ps[:],
                    qsT_t[c][:, j * 128:(j + 1) * 128],
                    kT_t[c][:, j * 128: j * 128 + 256],
                    start=(c == 0), stop=(c == 1))
            return ps

        def softmax(j, ps):
            # band mask on DVE, then exp (no max subtraction needed) with
            # free row sums; normalize on gpsimd; bf16 weights
            scm = wk.tile([128, 256], F32, tag="scm", name="scm")
            nc.vector.tensor_add(scm[:], ps[:],
                                 (mask_t0 if j == 0 else mask_std)[:])
            e_t = wk.tile([128, 256], BF16, tag="e", name="e_t")
            den = stat.tile([128, 1], F32, tag="den", name="den")
            nc.scalar.activation(e_t[:], scm[:], AF.Exp, accum_out=den[:])
            rec = stat.tile([128, 1], F32, tag="rec", name="rec")
            nc.vector.reciprocal(rec[:], den[:])
            w_t = wk.tile([128, 256], BF16, tag="w", name="w_t")
            nc.vector.tensor_scalar_mul(w_t[:], e_t[:], rec[:])
            # transpose both 128-wide halves on the PE (bf16, 1 cyc/row)
            pw = pwt.tile([128, 256], BF16, tag="pw", name="pw")
            nc.tensor.transpose(pw[:, 0:128], w_t[:, 0:128], ident)
            nc.tensor.transpose(pw[:, 128:256], w_t[:, 128:256], ident)
            nc.vector.tensor_copy(wTall[:, j * 256:(j + 1) * 256], pw[:])

        def pv(p):
            # pair PV in bf16: middle key block shared by both tiles
            # (256-wide moving), edge blocks 128-wide
            pc = pct.tile([128, 512], F32, tag="pc", name="pc")
            base = 2 * p * 256
            for h in range(2):
                hs = slice(h * 128, (h + 1) * 128)
                o = h * 256
                nc.tensor.matmul(          # m=1: key block 2p+1, both tiles
                    pc[:, o: o + 256],
                    kNv[:, 2 * p + 1, hs],
                    wTall[:, base + 128: base + 384],
                    start=True, stop=False)
                nc.tensor.matmul(          # m=0: key block 2p, tile 2p only
                    pc[:, o: o + 128],
                    kNv[:, 2 * p, hs],
                    wTall[:, base: base + 128],
                    start=False, stop=False, skip_group_check=True)
                nc.tensor.matmul(          # m=2: key block 2p+2, tile 2p+1
                    pc[:, o + 128: o + 256],
                    kNv[:, 2 * p + 2, hs],
                    wTall[:, base + 384: base + 512],
                    start=False, stop=True, skip_group_check=True)
            # scatter h-chunks into c8 slots (stride TPQ), cast to fp8
            nc.vector.tensor_copy(
                c8v[:, 0:2, 2 * p * 128: 2 * p * 128 + 256],
                pc[:].rearrange("p (b x) -> p b x", x=256))

        def p2(nb, half=None):
            # enhT[f, t'] = tanh(W_enh.T @ [cT; qT] + b_enh)
            # q-half exact fp32r; c-half one fp8 DoubleRow matmul
            if half is None:
                t0, tw = nb * 512, 512
            else:
                t0, tw = nb * 512 + half * 256, 256
            for f in range(2):
                pe_ = pmm.tile([128, tw], F32, tag="mm", name="pe_")
                for d in range(2):
                    nc.tensor.matmul(
                        pe_[:],
                        weq[d][:, f * 128:(f + 1) * 128],
                        qT_t[d][:, t0:t0 + tw],
                        start=(d == 0), stop=False)
                nc.tensor.matmul(
                    pe_[:],
                    wecv[:, 0:2, f * 128:(f + 1) * 128],
                    c8v[:, 0:2, t0:t0 + tw],
                    start=False, stop=True, perf_mode=DRM)
                nc.scalar.activation(
                    env[:, f:f + 1, t0:t0 + tw],
                    pe_[:].rearrange("p (b x) -> p b x", x=tw),
                    AF.Tanh, bias=bet[f][:, 0:1])

        def p3(j):
            # z = enh @ W_mask.T + b_mask ; out = 0.5*tanh(z/2)+0.5
            pm = pmm.tile([128, OPAD], F32, tag="mm", name="pm")
            for f in range(2):
                nc.tensor.matmul(
                    pm[:],
                    env[:, f:f + 1, j * 128:(j + 1) * 128],
                    wmt[f][:],
                    start=(f == 0), stop=(f == 1))
            z_t = wk.tile([128, OPAD], F32, tag="z", name="z_t")
            nc.vector.tensor_add(z_t[:], pm[:], bm_t[:])
            o_t = wk.tile([128, OPAD], F32, tag="o", name="o_t")
            nc.scalar.activation(o_t[:], z_t[:], AF.Tanh, scale=0.5)
            o2_t = wk.tile([128, OPAD], F32, tag="o2", name="o2_t")
            nc.gpsimd.tensor_scalar(o2_t[:], o_t[:], 0.5, 0.5,
                                    op0=ALU.mult, op1=ALU.add)
            rows = min(128, T - j * 128)
            nc.sync.dma_start(out[j * 128: j * 128 + rows, :],
                              o2_t[0:rows, 0:F_OUT])

        # ---- attention loop, software-pipelined with lag 2 ----
        # P3 of group g runs spread over group g+1's iterations
        LAG = 2
        p0(0)
        ps_q = {jj: scores(jj) for jj in range(LAG)}
        for nb in range(1, 4):
            p0(nb)
        pending_p3 = []
        for j in range(NT):
            if j % 4 == 0 and j > 0:
                nb = j // 4 - 1
                p2(nb)
                pending_p3.extend(range(nb * 4, nb * 4 + 4))
            if j + LAG < NT:
                ps_q[j + LAG] = scores(j + LAG)
            softmax(j, ps_q.pop(j))
            if j == 14:
                p2(3, half=0)      # tiles 12,13 (pair 6 stored at j=13)
                p3(12)
                p3(13)
            if pending_p3:
                p3(pending_p3.pop(0))
            if j % 2 == 1:
                pv(j // 2)
        p2(3, half=1)              # tiles 14,15
        for jj in [14, 15]:
            p3(jj)

    return nc


def _prep_shared(W_score, W_enh, b_enh, W_mask, b_mask):
    WsT = np.ascontiguousarray(W_score.T.astype(np.float32))        # [h, g]
    We = np.ascontiguousarray(W_enh.T.astype(np.float32))           # [d, f]
    WeqT = np.ascontiguousarray(We[H:])                             # [d', f]
    Wec8 = np.ascontiguousarray(
        We[:H].reshape(2, 128, H).transpose(1, 0, 2).reshape(128, 2 * H)
    ).astype(NP_FP8)
    Wm = np.zeros((H, OPAD), np.float32)                            # [f, o]
    Wm[:, :F_OUT] = W_mask.T.astype(np.float32)
    WmT16 = Wm.astype(NP_BF16)
    be = np.ascontiguousarray(b_enh.astype(np.float32).reshape(H, 1))
    bmv = np.zeros((128, OPAD), np.float32)
    bmv[:, :F_OUT] = b_mask.astype(np.float32)[None, :]
    return WsT, WeqT, Wec8, WmT16, be, bmv


def make_in_maps(k, q, W_score, W_enh, b_enh, W_mask, b_mask):
    k = np.asarray(k, np.float32)
    q = np.asarray(q, np.float32)
    WsT, WeqT, Wec8, WmT16, be, bmv = _prep_shared(
        np.asarray(W_score, np.float32), np.asarray(W_enh, np.float32),
        np.asarray(b_enh, np.float32), np.asarray(W_mask, np.float32),
        np.asarray(b_mask, np.float32))
    in_maps = []
    for b in range(N_CORES):
        kb = np.zeros((TPK, H), np.float32)
        kb[128:128 + T] = k[b]
        qb = np.zeros((TPQ, H), np.float32)
        qb[:T] = q[b]
        kN16 = np.ascontiguousarray(
            kb.reshape(NKB, 128, H).transpose(1, 0, 2).reshape(128, NKB * 256)
        ).astype(NP_BF16)
        in_maps.append({
            "kT": np.ascontiguousarray(kb.T),
            "qT": np.ascontiguousarray(qb.T),
            "kN16": kN16,
            "WsT": WsT, "WeqT": WeqT, "Wec8": Wec8, "WmT16": WmT16,
            "be": be, "bm": bmv,
        })
    return in_maps


def get_nc():
    if "nc" not in _CACHE:
        nc = build_nc()
        nc.finalize()
        _CACHE["nc"] = nc
    return _CACHE["nc"]


def kernel(k, q, W_score, W_enh, b_enh, W_mask, b_mask):
    in_maps = make_in_maps(k, q, W_score, W_enh, b_enh, W_mask, b_mask)
    res = run_bass_kernel_spmd(get_nc(), in_maps, list(range(N_CORES)))
    return np.stack([r["out"] for r in res.results], 0)
